# revision 16
# baseline (speedup 1.0000x reference)
"""Trainium2 Bass kernel for the cosine-similarity triplet criterion.

The reference loss loop overwrites `loss` every iteration, so only the LAST
anchor's loss survives: out = ((cos(a,p) - mean_m cos(a,n_m)) - 1)^2 for
a = batch[anchors[-1]], p = batch[positives[-1]], n = batch[negatives[-1]].

Host side gathers the 2+M relevant rows of `batch` (the sharding step); the
device computes row norms, the eps-clamped cosines, the negative mean, and the
squared loss. The tiny computation is replicated on all 8 cores (data-parallel
degenerate case: only one anchor's loss survives dead-code elimination).
"""

import numpy as np

_CACHE: dict = {}


def _build(M: int, D: int):
    import concourse.bacc as bacc
    import concourse.bass as bass
    import concourse.tile as tile
    from concourse import mybir

    R = 2 + M  # anchor, positive, M negatives
    f32 = mybir.dt.float32

    # Bacc (not raw Bass): its finalize() runs the backend passes that split
    # multi-semaphore waits into event-semaphore chains (TRN2 allows only one
    # wait per instruction) and legalize raw-ISA instruction encodings.
    nc = bacc.Bacc("TRN2", target_bir_lowering=False)
    # Packed input: [row data | anchor-row broadcast | mask weight].
    # Cols 0..D-1: the gathered rows; cols D..2D-1: the anchor row replicated
    # to every partition; col 2D: reduction-mask weight (+1 positive,
    # -1/M negatives, 0 anchor). One tensor -> one DMA -> one DMA semaphore,
    # which keeps the kernel-tail drain under the codegen sync-wait limit.
    rowsm = nc.dram_tensor("rowsm", [R, 2 * D + 1], f32, kind="ExternalInput")
    loss = nc.dram_tensor("loss", [1, 1], f32, kind="ExternalOutput")

    with tile.TileContext(nc) as tc:
        with (
            tc.tile_pool(name="pool", bufs=1) as pool,
            tc.tile_pool(name="psum", bufs=1, space="PSUM") as psum,
        ):
            xm = pool.tile([R, 2 * D + 1], f32)
            nc.sync.dma_start(out=xm, in_=rowsm[:, :])
            x = xm[:, 0:D]
            ab = xm[:, D : 2 * D]
            mask = xm[:, 2 * D : 2 * D + 1]

            # dots[i] = <x[i], a>, ss[i] = <x[i], x[i]>  (fused mul + row-sum)
            prod = pool.tile([R, D], f32)
            dots = pool.tile([R, 1], f32)
            nc.vector.scalar_tensor_tensor(
                out=prod, in0=x, scalar=1.0, in1=ab,
                op0=mybir.AluOpType.mult, op1=mybir.AluOpType.mult, accum_out=dots,
            )
            sq = pool.tile([R, D], f32)
            ss = pool.tile([R, 1], f32)
            nc.vector.scalar_tensor_tensor(
                out=sq, in0=x, scalar=1.0, in1=x,
                op0=mybir.AluOpType.mult, op1=mybir.AluOpType.mult, accum_out=ss,
            )

            # inv[i] = 1 / max(sqrt(ss[i]), eps)   (torch CosineSimilarity eps)
            norm = pool.tile([R, 1], f32)
            nc.scalar.activation(out=norm, in_=ss, func=mybir.ActivationFunctionType.Sqrt)
            nc.vector.tensor_scalar_max(out=norm, in0=norm, scalar1=1e-8)
            inv = pool.tile([R, 1], f32)
            nc.vector.reciprocal(out=inv, in_=norm)

            # t[i] = dots[i] * inv[i]  (cosine up to the anchor's inv factor)
            t = pool.tile([R, 1], f32)
            nc.vector.tensor_mul(out=t, in0=dots, in1=inv)

            # tw[i] = t[i] * mask[i]; sum over partitions = (cp - cn) / inv_a.
            # The sum is a 1-column PE matmul: ones.T @ tw -> [1, 1] PSUM.
            tw = pool.tile([R, 1], f32)
            nc.vector.tensor_mul(out=tw, in0=t, in1=mask)
            ones = pool.tile([R, 1], f32)
            nc.vector.memset(ones, 1.0)
            ps = psum.tile([1, 1], f32)
            nc.tensor.matmul(ps, tw, ones, start=True, stop=True)

            # loss = (ps * inv_a - 1)^2
            d1 = pool.tile([1, 1], f32)
            nc.vector.tensor_scalar(
                out=d1, in0=ps[0:1, 0:1], scalar1=inv[0:1, 0:1], scalar2=-1.0,
                op0=mybir.AluOpType.mult, op1=mybir.AluOpType.add,
            )
            lt = pool.tile([1, 1], f32)
            nc.vector.tensor_mul(out=lt, in0=d1, in1=d1)
            nc.sync.dma_start(out=loss[:, :], in_=lt)

    nc.finalize()
    return nc


def _run(inputs, trace: bool = False):
    from concourse import bass_utils

    batch = np.ascontiguousarray(np.asarray(inputs["batch"]), dtype=np.float32)
    anchors = np.asarray(inputs["anchors"])
    positives = np.asarray(inputs["positives"])
    negatives = np.asarray(inputs["negatives"])

    D = batch.shape[1]
    M = negatives.shape[1]
    a = int(anchors[-1])
    p = int(positives[-1])
    negs = negatives[-1].astype(np.int64)
    rows = np.ascontiguousarray(
        np.concatenate([batch[a : a + 1], batch[p : p + 1], batch[negs]], axis=0),
        dtype=np.float32,
    )

    maskv = np.zeros((2 + M, 1), dtype=np.float32)
    maskv[1, 0] = 1.0
    maskv[2:, 0] = -1.0 / M
    rowsm = np.ascontiguousarray(
        np.concatenate(
            [rows, np.broadcast_to(rows[0:1, :], rows.shape), maskv], axis=1
        ),
        dtype=np.float32,
    )

    key = (M, D)
    if key not in _CACHE:
        _CACHE[key] = _build(M, D)
    nc = _CACHE[key]

    n_cores = 8
    res = bass_utils.run_bass_kernel_spmd(
        nc,
        [{"rowsm": rowsm}] * n_cores,
        core_ids=list(range(n_cores)),
        trace=trace,
    )
    out = np.asarray(res.results[0]["loss"], dtype=np.float32).reshape(1, 1)
    return out, res


def kernel(**inputs) -> np.ndarray:
    out, _ = _run(inputs)
    return out
